# revision 14
# baseline (speedup 1.0000x reference)
"""Weighted cross-entropy loss (nn_CustomCrossEntropyLoss) on 8 Trainium2 NeuronCores.

Data-parallel over N=4M rows.  Sharding strategy (ours to choose): the host
permutes rows so that every row slot's TARGET CLASS is a static function of its
position — rows are grouped by target class into fixed-size per-partition
segments (host does no arithmetic on values, only placement + bf16 cast).
Each per-partition tile of F rows = 9 segments of F_c rows, segment c holding
rows with target class c.  The target-logit gather then degenerates to a static
strided access pattern (a "diagonal" AP over the class-major logit tile), and
the per-row weight w[t] is a static per-position vector (uploaded, 0 on pads).

Per tile [128 x F rows], logits class-major X[p, 9, F] bf16:
  ACT:  E = exp(X)                               [p, 9F]
  PE :  S = sum_c E_c   (9 identity matmuls accumulating in PSUM, f32)
  ACT:  L = ln(S) -> bf16                        [p, F]
  DVE:  D  = L - X[diag]        (TT, 2x bf16)    per-row target logit via AP
        LW = D * wvec           (TT, 2x)         = per-row loss, wvec=0 on pads
        loss_sum += LW          (TS accum, 4x)
        count    += (LW > 1e-16) (TS accum, 4x)  literal reference check
Host sums the 8x128 partial [loss_sum, count] pairs.

Cost model budget/core: ACT ~35us (exp+ln, bottleneck), PE ~29us, DMA ~28us
(10MB bf16), DVE ~10us.
"""

import sys

if "/opt/trn_rl_repo" not in sys.path:
    sys.path.insert(0, "/opt/trn_rl_repo")

from contextlib import ExitStack

import numpy as np
import ml_dtypes

import concourse.bass as bass
import concourse.mybir as mybir
from concourse.ap import AP
from concourse.bass_utils import run_bass_kernel_spmd

F32 = mybir.dt.float32
BF16 = mybir.dt.bfloat16
AF = mybir.ActivationFunctionType
ALU = mybir.AluOpType
BF = ml_dtypes.bfloat16

N = 4_000_000
C = 9
NCORES = 8
P = 128
T = 5            # tiles per core
PADX = -3.0      # pad-row logit (harmless through exp; wvec=0 excludes pads)
PF = 1024        # PSUM slot stride (f32), bank-aligned
H = 512          # matmul moving-dim split (max 512)
CH0 = 4          # classes in exp/dma chunk A (chunk B = C - CH0)

W = [0.03203128, 0.12453853, 0.12360233, 0.12430233, 0.1118631,
     0.11928928, 0.12498565, 0.12078846, 0.11859904]

_CACHED = {}


def _build_nc(Fc):
    F = C * Fc
    nc = bass.Bass()
    x = nc.declare_dram_parameter("x", [P, T, C * F], BF16, isOutput=False)
    wv = nc.declare_dram_parameter("wv", [P, T, F], BF16, isOutput=False)
    ident = nc.declare_dram_parameter("ident", [P, P], BF16, isOutput=False)
    y = nc.declare_dram_parameter("y", [P, 2], F32, isOutput=True)

    with ExitStack() as ctx:
        e = ctx.enter_context
        Xb = e(nc.sbuf_tensor([P, 3, C * F], BF16))
        Eb = e(nc.sbuf_tensor([P, 2, C * F], BF16))
        Wv = e(nc.sbuf_tensor([P, 3, F], BF16))
        Lb = e(nc.sbuf_tensor([P, 2, F], BF16))
        Db = e(nc.sbuf_tensor([P, F], BF16))
        LWb = e(nc.sbuf_tensor([P, F], BF16))
        IDb = e(nc.sbuf_tensor([P, P], BF16))
        losscol = e(nc.sbuf_tensor([P, T], F32))
        ccol = e(nc.sbuf_tensor([P, T], F32))
        outb = e(nc.sbuf_tensor([P, 2], F32))
        Sp = e(nc.psum_tensor([P, 2, PF], F32))
        IDS = e(nc.semaphore())
        ES = e(nc.semaphore())   # exp chunks done: 2 per tile
        SM = e(nc.semaphore())   # S-matmuls(k) done -> k+1
        LS = e(nc.semaphore())   # ln(k) done -> k+1
        VD = e(nc.semaphore())   # DVE(k) consumed -> k+1
        FIN = e(nc.semaphore())
        DOUT = e(nc.semaphore())
        dx = [e(nc.semaphore(name=f"dx{_k}")) for _k in range(T)]

        # Per-tile exp/DMA class-chunking: fine-grained on the first tile so
        # the first exp starts as soon as one class has landed (pipeline
        # fill), fine-grained at the end of the last tile so the final
        # matmul group trails the final exp chunk closely (pipeline drain).
        chunks = []
        for k in range(T):
            if k == 0:
                ck = [(0, 1), (1, 3), (3, 6), (6, C)]
            elif k == T - 1:
                ck = [(0, 4), (4, 7), (7, C)]
            else:
                ck = [(0, C)]
            chunks.append(ck)
        es_base = [sum(len(chunks[j]) for j in range(k)) for k in range(T)]

        def diag_ap(s):
            # X[p, c*F + c*Fc + j] for c in 0..8, j in 0..Fc: target-class
            # logit of row slot (c, j) in the class-sorted layout.
            base = Xb[:, s, :]
            return AP(
                tensor=base.tensor,
                offset=base.offset,
                ap=[list(base.ap[0]), [F + Fc, C], [1, Fc]],
            )

        with nc.Block() as block:

            @block.sync
            def _(sync):
                for k in range(T):
                    s = k % 3
                    if k >= 3:
                        sync.wait_ge(VD, k - 2)  # Xb/Wv slot consumed
                    for c0, c1 in chunks[k]:
                        sync.dma_start(
                            Xb[:, s, c0 * F : c1 * F], x[:, k, c0 * F : c1 * F]
                        ).then_inc(dx[k], 16)
                    sync.dma_start(Wv[:, s, :], wv[:, k, :]).then_inc(dx[k], 16)
                    if k == 0:
                        sync.dma_start(IDb[:, :], ident[:, :]).then_inc(IDS, 16)
                sync.wait_ge(DOUT, 16)

            @block.scalar
            def _(scalar):
                def ln(j):
                    sj = j % 2
                    scalar.wait_ge(SM, j + 1)
                    if j >= 2:
                        scalar.wait_ge(VD, j - 1)  # Lb slot free
                    scalar.activation(
                        Lb[:, sj, :], Sp[:, sj, 0:F], AF.Ln
                    ).then_inc(LS, 1)

                for k in range(T):
                    s3 = k % 3
                    s = k % 2
                    for i, (c0, c1) in enumerate(chunks[k]):
                        scalar.wait_ge(dx[k], 16 * (i + 1))
                        if i == 0 and k >= 2:
                            scalar.wait_ge(SM, k - 1)  # Eb slot read by mms(k-2)
                        scalar.activation(
                            Eb[:, s, c0 * F : c1 * F], Xb[:, s3, c0 * F : c1 * F],
                            AF.Exp,
                        ).then_inc(ES, 1)
                        if i == 0 and k >= 1:
                            ln(k - 1)
                ln(T - 1)

            @block.tensor
            def _(tensor):
                tensor.wait_ge(IDS, 16)
                halves = ((0, H), (H, F)) if F > H else ((0, F),)
                for k in range(T):
                    s = k % 2
                    for i, (c0, c1) in enumerate(chunks[k]):
                        tensor.wait_ge(ES, es_base[k] + i + 1)
                        if i == 0 and k >= 2:
                            tensor.wait_ge(LS, k - 1)  # Sp slot read by ln(k-2)
                        for h0, h1 in halves:
                            for c in range(c0, c1):
                                mm = tensor.matmul(
                                    Sp[:, s, h0:h1],
                                    IDb[:, :],
                                    Eb[:, s, c * F + h0 : c * F + h1],
                                    start=(c == 0),
                                    stop=(c == C - 1),
                                )
                    mm.then_inc(SM, 1)

            @block.gpsimd
            def _(gpsimd):
                gpsimd.wait_ge(FIN, 1)
                gpsimd.dma_start(y[:, :], outb[:, :]).then_inc(DOUT, 16)

            @block.vector
            def _(vector):
                for k in range(T):
                    s3 = k % 3
                    s = k % 2
                    vector.wait_ge(LS, k + 1)
                    vector.wait_ge(dx[k], 16 * (len(chunks[k]) + 1))  # wvec arrival
                    l3 = Lb[:, s, :].rearrange("p (c f) -> p c f", c=C)
                    d3 = Db[:, :].rearrange("p (c f) -> p c f", c=C)
                    vector.tensor_tensor(d3, l3, diag_ap(s3), ALU.subtract)
                    vector.tensor_tensor(LWb[:, :], Db[:, :], Wv[:, s3, :], ALU.mult)
                    vector.tensor_scalar(
                        Db[:, :], LWb[:, :], 0.0, 0.0, ALU.add, ALU.add,
                        accum_out=losscol[:, k : k + 1],
                    )
                    vector.tensor_scalar(
                        Db[:, :], LWb[:, :], 1e-16, 0.0, ALU.is_gt, ALU.add,
                        accum_out=ccol[:, k : k + 1],
                    ).then_inc(VD, 1)
                vector.tensor_reduce(
                    outb[:, 0:1], losscol[:, :], axis=mybir.AxisListType.X, op=ALU.add
                )
                vector.tensor_reduce(
                    outb[:, 1:2], ccol[:, :], axis=mybir.AxisListType.X, op=ALU.add
                ).then_inc(FIN, 1)

    return nc


def _get_nc(Fc=None):
    if Fc is None:
        Fc = _CACHED.get("Fc", 87)
    if _CACHED.get("Fc") != Fc:
        _CACHED["nc"] = _build_nc(Fc)
        _CACHED["Fc"] = Fc
    return _CACHED["nc"]


def _prep_inputs(logits, target):
    logits = np.asarray(logits, dtype=np.float32)
    target = np.asarray(target).astype(np.int64)
    counts = np.bincount(target, minlength=C)
    Fc = int(-(-counts.max() // (P * T * NCORES)))
    F = C * Fc
    CAP = P * T * NCORES * Fc

    order = np.argsort(target, kind="stable")
    A = np.full((C, CAP), N, dtype=np.int64)
    pos = 0
    for c in range(C):
        A[c, : counts[c]] = order[pos : pos + counts[c]]
        pos += counts[c]
    # [C, cores, P, T, Fc] -> [cores, P, T, Cseg, Fc]
    Ar = A.reshape(C, NCORES, P, T, Fc).transpose(1, 2, 3, 0, 4)

    logits_ext = np.concatenate(
        [logits, np.full((1, C), PADX, dtype=np.float32)], axis=0
    )
    Xg = logits_ext[Ar]                      # [cores, P, T, Cseg, Fc, Cdim]
    Xc = Xg.transpose(0, 1, 2, 5, 3, 4)      # [cores, P, T, Cdim, Cseg, Fc]
    xsh = np.ascontiguousarray(Xc).astype(BF).reshape(NCORES, P, T, C * F)

    wvec = np.where(
        Ar < N, np.array(W, dtype=np.float32)[None, None, None, :, None], 0.0
    ).astype(BF)                             # [cores, P, T, Cseg, Fc]
    wsh = wvec.reshape(NCORES, P, T, F)

    id_np = np.eye(P, dtype=BF)
    return Fc, [
        {"x": xsh[i], "wv": wsh[i], "ident": id_np} for i in range(NCORES)
    ]


def run_on_hw(logits, target, trace=False):
    Fc, in_maps = _prep_inputs(logits, target)
    nc = _get_nc(Fc)
    res = run_bass_kernel_spmd(nc, in_maps, core_ids=list(range(NCORES)), trace=trace)
    ys = np.stack([res.results[i]["y"] for i in range(NCORES)])  # [8, 128, 2]
    loss_sum = ys[:, :, 0].sum(dtype=np.float64)
    cnt = ys[:, :, 1].sum(dtype=np.float64)
    return loss_sum, cnt, res


def kernel(logits, target, class_weights=None):
    loss_sum, cnt, _ = run_on_hw(logits, target)
    out1 = np.float32(loss_sum / (cnt + 1e-16))
    out2 = np.float32(loss_sum / N)
    return (out1, out2)


if __name__ == "__main__":
    rng = np.random.default_rng(0)
    lg = rng.standard_normal((N, C), dtype=np.float32)
    tg = rng.integers(0, C, size=(N,)).astype(np.int64)
    print(kernel(lg, tg))


# revision 15
# speedup vs baseline: 1.0080x; 1.0080x over previous
"""Weighted cross-entropy loss (nn_CustomCrossEntropyLoss) on 8 Trainium2 NeuronCores.

Data-parallel over N=4M rows.  Sharding strategy (ours to choose): the host
permutes rows so that every row slot's TARGET CLASS is a static function of its
position — rows are grouped by target class into fixed-size per-partition
segments (host does no arithmetic on values, only placement + bf16 cast).
Each per-partition tile of F rows = 9 segments of F_c rows, segment c holding
rows with target class c.  The target-logit gather then degenerates to a static
strided access pattern (a "diagonal" AP over the class-major logit tile), and
the per-row weight w[t] is a static per-position vector (uploaded, 0 on pads).

Per tile [128 x F rows], logits class-major X[p, 9, F] bf16:
  ACT:  E = exp(X)                               [p, 9F]
  PE :  S = sum_c E_c   (9 identity matmuls accumulating in PSUM, f32)
  ACT:  L = ln(S) -> bf16                        [p, F]
  DVE:  D  = L - X[diag]        (TT, 2x bf16)    per-row target logit via AP
        LW = D * wvec           (TT, 2x)         = per-row loss, wvec=0 on pads
        loss_sum += LW          (TS accum, 4x)
        count    += (LW > 1e-16) (TS accum, 4x)  literal reference check
Host sums the 8x128 partial [loss_sum, count] pairs.

Cost model budget/core: ACT ~35us (exp+ln, bottleneck), PE ~29us, DMA ~28us
(10MB bf16), DVE ~10us.
"""

import sys

if "/opt/trn_rl_repo" not in sys.path:
    sys.path.insert(0, "/opt/trn_rl_repo")

from contextlib import ExitStack

import numpy as np
import ml_dtypes

import concourse.bass as bass
import concourse.mybir as mybir
from concourse.ap import AP
from concourse.bass_utils import run_bass_kernel_spmd

F32 = mybir.dt.float32
BF16 = mybir.dt.bfloat16
AF = mybir.ActivationFunctionType
ALU = mybir.AluOpType
BF = ml_dtypes.bfloat16

N = 4_000_000
C = 9
NCORES = 8
P = 128
T = 5            # tiles per core
PADX = -3.0      # pad-row logit (harmless through exp; wvec=0 excludes pads)
PF = 1024        # PSUM slot stride (f32), bank-aligned
H = 512          # matmul moving-dim split (max 512)
CH0 = 4          # classes in exp/dma chunk A (chunk B = C - CH0)

W = [0.03203128, 0.12453853, 0.12360233, 0.12430233, 0.1118631,
     0.11928928, 0.12498565, 0.12078846, 0.11859904]

_CACHED = {}


def _build_nc(Fc):
    F = C * Fc
    nc = bass.Bass()
    x = nc.declare_dram_parameter("x", [P, T, C * F], BF16, isOutput=False)
    wv = nc.declare_dram_parameter("wv", [P, T, F], BF16, isOutput=False)
    ident = nc.declare_dram_parameter("ident", [P, P], BF16, isOutput=False)
    y = nc.declare_dram_parameter("y", [P, 2], F32, isOutput=True)

    with ExitStack() as ctx:
        e = ctx.enter_context
        Xb = e(nc.sbuf_tensor([P, 3, C * F], BF16))
        Eb = e(nc.sbuf_tensor([P, 2, C * F], BF16))
        Wv = e(nc.sbuf_tensor([P, 3, F], BF16))
        Lb = e(nc.sbuf_tensor([P, 2, F], BF16))
        Db = e(nc.sbuf_tensor([P, F], BF16))
        LWb = e(nc.sbuf_tensor([P, F], BF16))
        IDb = e(nc.sbuf_tensor([P, P], BF16))
        losscol = e(nc.sbuf_tensor([P, T], F32))
        ccol = e(nc.sbuf_tensor([P, T], F32))
        outb = e(nc.sbuf_tensor([P, 2], F32))
        Sp = e(nc.psum_tensor([P, 2, PF], F32))
        IDS = e(nc.semaphore())
        ES = e(nc.semaphore())   # exp chunks done: 2 per tile
        SM = e(nc.semaphore())   # S-matmuls(k) done -> k+1
        LS = e(nc.semaphore())   # ln(k) done -> k+1
        VD = e(nc.semaphore())   # DVE(k) consumed -> k+1
        FIN = e(nc.semaphore())
        DOUT = e(nc.semaphore())
        dx = [e(nc.semaphore(name=f"dx{_k}")) for _k in range(T)]

        # Per-tile exp/DMA class-chunking: fine-grained on the first tile so
        # the first exp starts as soon as one class has landed (pipeline
        # fill), fine-grained at the end of the last tile so the final
        # matmul group trails the final exp chunk closely (pipeline drain).
        chunks = []
        for k in range(T):
            if k == 0:
                ck = [(0, 1), (1, 3), (3, 6), (6, C)]
            elif k == T - 1:
                ck = [(0, 4), (4, 7), (7, C)]
            else:
                ck = [(0, C)]
            chunks.append(ck)
        es_base = [sum(len(chunks[j]) for j in range(k)) for k in range(T)]

        def diag_ap(s):
            # X[p, c*F + c*Fc + j] for c in 0..8, j in 0..Fc: target-class
            # logit of row slot (c, j) in the class-sorted layout.
            base = Xb[:, s, :]
            return AP(
                tensor=base.tensor,
                offset=base.offset,
                ap=[list(base.ap[0]), [F + Fc, C], [1, Fc]],
            )

        with nc.Block() as block:

            @block.sync
            def _(sync):
                for k in range(T):
                    s = k % 3
                    if k >= 3:
                        sync.wait_ge(VD, k - 2)  # Xb/Wv slot consumed
                    for c0, c1 in chunks[k]:
                        sync.dma_start(
                            Xb[:, s, c0 * F : c1 * F], x[:, k, c0 * F : c1 * F]
                        ).then_inc(dx[k], 16)
                    sync.dma_start(Wv[:, s, :], wv[:, k, :]).then_inc(dx[k], 16)
                    if k == 0:
                        sync.dma_start(IDb[:, :], ident[:, :]).then_inc(IDS, 16)
                sync.wait_ge(FIN, 1)
                sync.dma_start(y[:, :], outb[:, :]).then_inc(DOUT, 16)
                sync.wait_ge(DOUT, 16)

            @block.scalar
            def _(scalar):
                def ln(j):
                    sj = j % 2
                    scalar.wait_ge(SM, j + 1)
                    if j >= 2:
                        scalar.wait_ge(VD, j - 1)  # Lb slot free
                    scalar.activation(
                        Lb[:, sj, :], Sp[:, sj, 0:F], AF.Ln
                    ).then_inc(LS, 1)

                for k in range(T):
                    s3 = k % 3
                    s = k % 2
                    for i, (c0, c1) in enumerate(chunks[k]):
                        scalar.wait_ge(dx[k], 16 * (i + 1))
                        if i == 0 and k >= 2:
                            scalar.wait_ge(SM, k - 1)  # Eb slot read by mms(k-2)
                        scalar.activation(
                            Eb[:, s, c0 * F : c1 * F], Xb[:, s3, c0 * F : c1 * F],
                            AF.Exp,
                        ).then_inc(ES, 1)
                        if i == 0 and k >= 1:
                            ln(k - 1)
                ln(T - 1)

            @block.tensor
            def _(tensor):
                tensor.wait_ge(IDS, 16)
                halves = ((0, H), (H, F)) if F > H else ((0, F),)
                for k in range(T):
                    s = k % 2
                    for i, (c0, c1) in enumerate(chunks[k]):
                        tensor.wait_ge(ES, es_base[k] + i + 1)
                        if i == 0 and k >= 2:
                            tensor.wait_ge(LS, k - 1)  # Sp slot read by ln(k-2)
                        for h0, h1 in halves:
                            for c in range(c0, c1):
                                mm = tensor.matmul(
                                    Sp[:, s, h0:h1],
                                    IDb[:, :],
                                    Eb[:, s, c * F + h0 : c * F + h1],
                                    start=(c == 0),
                                    stop=(c == C - 1),
                                )
                    mm.then_inc(SM, 1)

            @block.vector
            def _(vector):
                for k in range(T):
                    s3 = k % 3
                    s = k % 2
                    vector.wait_ge(LS, k + 1)
                    vector.wait_ge(dx[k], 16 * (len(chunks[k]) + 1))  # wvec arrival
                    l3 = Lb[:, s, :].rearrange("p (c f) -> p c f", c=C)
                    d3 = Db[:, :].rearrange("p (c f) -> p c f", c=C)
                    vector.tensor_tensor(d3, l3, diag_ap(s3), ALU.subtract)
                    vector.tensor_tensor(LWb[:, :], Db[:, :], Wv[:, s3, :], ALU.mult)
                    vector.tensor_scalar(
                        Db[:, :], LWb[:, :], 0.0, 0.0, ALU.add, ALU.add,
                        accum_out=losscol[:, k : k + 1],
                    )
                    vector.tensor_scalar(
                        Db[:, :], LWb[:, :], 1e-16, 0.0, ALU.is_gt, ALU.add,
                        accum_out=ccol[:, k : k + 1],
                    ).then_inc(VD, 1)
                vector.tensor_reduce(
                    outb[:, 0:1], losscol[:, :], axis=mybir.AxisListType.X, op=ALU.add
                )
                vector.tensor_reduce(
                    outb[:, 1:2], ccol[:, :], axis=mybir.AxisListType.X, op=ALU.add
                ).then_inc(FIN, 1)

    return nc


def _get_nc(Fc=None):
    if Fc is None:
        Fc = _CACHED.get("Fc", 87)
    if _CACHED.get("Fc") != Fc:
        _CACHED["nc"] = _build_nc(Fc)
        _CACHED["Fc"] = Fc
    return _CACHED["nc"]


def _prep_inputs(logits, target):
    logits = np.asarray(logits, dtype=np.float32)
    target = np.asarray(target).astype(np.int64)
    counts = np.bincount(target, minlength=C)
    Fc = int(-(-counts.max() // (P * T * NCORES)))
    F = C * Fc
    CAP = P * T * NCORES * Fc

    order = np.argsort(target, kind="stable")
    A = np.full((C, CAP), N, dtype=np.int64)
    pos = 0
    for c in range(C):
        A[c, : counts[c]] = order[pos : pos + counts[c]]
        pos += counts[c]
    # [C, cores, P, T, Fc] -> [cores, P, T, Cseg, Fc]
    Ar = A.reshape(C, NCORES, P, T, Fc).transpose(1, 2, 3, 0, 4)

    logits_ext = np.concatenate(
        [logits, np.full((1, C), PADX, dtype=np.float32)], axis=0
    )
    Xg = logits_ext[Ar]                      # [cores, P, T, Cseg, Fc, Cdim]
    Xc = Xg.transpose(0, 1, 2, 5, 3, 4)      # [cores, P, T, Cdim, Cseg, Fc]
    xsh = np.ascontiguousarray(Xc).astype(BF).reshape(NCORES, P, T, C * F)

    wvec = np.where(
        Ar < N, np.array(W, dtype=np.float32)[None, None, None, :, None], 0.0
    ).astype(BF)                             # [cores, P, T, Cseg, Fc]
    wsh = wvec.reshape(NCORES, P, T, F)

    id_np = np.eye(P, dtype=BF)
    return Fc, [
        {"x": xsh[i], "wv": wsh[i], "ident": id_np} for i in range(NCORES)
    ]


def run_on_hw(logits, target, trace=False):
    Fc, in_maps = _prep_inputs(logits, target)
    nc = _get_nc(Fc)
    res = run_bass_kernel_spmd(nc, in_maps, core_ids=list(range(NCORES)), trace=trace)
    ys = np.stack([res.results[i]["y"] for i in range(NCORES)])  # [8, 128, 2]
    loss_sum = ys[:, :, 0].sum(dtype=np.float64)
    cnt = ys[:, :, 1].sum(dtype=np.float64)
    return loss_sum, cnt, res


def kernel(logits, target, class_weights=None):
    loss_sum, cnt, _ = run_on_hw(logits, target)
    out1 = np.float32(loss_sum / (cnt + 1e-16))
    out2 = np.float32(loss_sum / N)
    return (out1, out2)


if __name__ == "__main__":
    rng = np.random.default_rng(0)
    lg = rng.standard_normal((N, C), dtype=np.float32)
    tg = rng.integers(0, C, size=(N,)).astype(np.int64)
    print(kernel(lg, tg))
